# revision 45
# baseline (speedup 1.0000x reference)
"""Trainium2 kernel for nn_PolynomialLayer: out = [x, x_i*x_j (i<=j)] @ W.T + bias.

Data-parallel over batch across 8 NeuronCores. Each core:
  - receives x^T for its 1024-row batch shard ([128 feat, 1024 b]) plus 64
    partition-rotated copies (host np.roll; pure data movement),
  - builds the 8256 pairwise-product features on the vector engine as
    full-128-partition tensor_tensor multiplies (chunk d: xT * rot_d covers
    all index pairs with cyclic difference {d, 128-d}),
  - accumulates out^T[512, 1024] = sum_c Wc.T @ PTc on the tensor engine.
    The last M_PAIRS*2 cross chunks run as fp8e4 DoubleRow pairs (2 K-chunks
    per matmul pass, ~1.77x streaming rate); the rest stay bf16. The fp8
    fraction is sized so the fp8 quantization noise keeps the output rel-err
    under the accuracy budget.
  - bias is folded into the GEMM via an all-ones feature row in the padded
    d=64 chunk (its weights row carries the bias), so PSUM holds the final
    result, and
  - drains PSUM -> SBUF (bf16) -> DRAM per 128-row output group, pipelined.
Dummy warmup matmuls at kernel start lift the PE clock gate (HAM) to full
rate before the real stream begins.
The host pre-permutes/transposes the weight matrix so its column order
matches the on-chip feature-chunk layout.
"""

import os
import sys
import numpy as np

for _p in ("/opt/trn_rl_repo",):
    if os.path.isdir(_p) and _p not in sys.path:
        sys.path.append(_p)

B, D, NOUT = 8192, 128, 512
NCORES = 8
BC = B // NCORES            # 1024 batch rows per core
NCHUNK = 66                 # 1 linear + 1 squares + 64 rotation chunks
NROT = 64                   # rotation distances d=1..64
NB = BC // 512              # moving-operand chunks per core (2)
NN = NOUT // 128            # output partition chunks (4)

# fp8 DoubleRow pairs (2 cross chunks each), taken from the tail of the
# cross-chunk sequence. Error budget: rel_fro ~= 0.032 * sqrt(2*m*128/8640).
M_PAIRS = int(os.environ.get("POLY_M_PAIRS", "9"))
N_WARMUP = int(os.environ.get("POLY_WARMUP", "6"))
ROT_BODY = int(os.environ.get("POLY_ROT_BODY", "4"))
WB_BODY = int(os.environ.get("POLY_WB_BODY", "4"))
# 1: build the 64 partition-rotated copies of xT on-chip via SBUF->SBUF
# DMAs on the scalar HWDGE queue. Measured 2.4x SLOWER than streaming
# host-rolled copies from DRAM: each partition-shifted SBUF->SBUF dma_start
# costs ~1.5us of descriptor generation, 128 of them serializing on the
# queue. Kept for reference; default is host-rolled (0).
ONCHIP_ROT = int(os.environ.get("POLY_ONCHIP_ROT", "0"))

# chunk processing order: [linear, squares] + bf16 cross (with the d64+bias
# chunk placed mid-stream, where supply slack is ample — its extra memset
# dependency would otherwise eat into the DMA-ramp window) + fp8 pairs.
# cross chunks are c=2..65 <-> d=c-1; c65 (d=64, half-padded) carries the
# all-ones bias row so it must stay bf16 and precedes the fp8 tail.
_FP8_CHUNKS = list(range(65 - 2 * M_PAIRS, 65))          # 2m chunks, d<=63
_BF16_CROSS = list(range(2, 65 - 2 * M_PAIRS))           # d=1..63-2m
PROC = ([0, 1] + _BF16_CROSS[:18] + [65] + _BF16_CROSS[18:] + _FP8_CHUNKS)
CROSS_ORDER = [c for c in PROC if c >= 2]                # rot slot order
NBF = 3 + len(_BF16_CROSS)                               # bf16 weight chunks
BIAS_ROW = 64                                            # ones row in c65


def _ensure_axon_hooks_stub():
    """concourse's trace path imports antenv.axon_hooks; provide a stub if
    this image lacks it so an env-triggered trace degrades instead of
    crashing."""
    try:
        import antenv.axon_hooks  # noqa: F401
        return
    except Exception:
        pass
    try:
        import types
        import antenv
        m = types.ModuleType("antenv.axon_hooks")
        m._hook = None
        m.set_axon_ntff_profile_hook = lambda h: setattr(m, "_hook", h)
        m.get_axon_ntff_profile_hook = lambda: m._hook
        sys.modules["antenv.axon_hooks"] = m
        antenv.axon_hooks = m
    except Exception:
        pass


def _pair_index_map():
    """Map (chunk c, partition p) -> column index in the reference feature
    order (or -1 for padding).

    Reference order: [x_0..x_127] then pairs (i,j) i<=j in
    combinations_with_replacement order.
    Chunk layout: c=0 linear; c=1 squares; c=2..65 -> d=c-1 in 1..64 with
    (i,j) = sorted(p, (p+d) % 128); for d=64 only p<64 is valid.
    """
    idx = np.full((NCHUNK, D), -1, dtype=np.int64)
    off = 128 * np.arange(D) - (np.arange(D) * (np.arange(D) - 1)) // 2

    def pair_idx(i, j):
        return D + off[i] + (j - i)

    idx[0, :] = np.arange(D)
    p = np.arange(D)
    idx[1, :] = pair_idx(p, p)
    for d in range(1, NROT + 1):
        c = 1 + d
        q = (p + d) % D
        i = np.minimum(p, q)
        j = np.maximum(p, q)
        v = pair_idx(i, j)
        if d == NROT:
            v = np.where(p < 64, v, -1)
        idx[c, :] = v
    return idx


def _group_sizes(total, leading=(1, 1, 2, 4), body=6):
    """DMA group sizes: small leading groups so the pipeline starts fast."""
    sizes = []
    for s in leading:
        if sum(sizes) + s > total:
            break
        sizes.append(s)
    while sum(sizes) < total:
        sizes.append(min(body, total - sum(sizes)))
    return sizes


_nc_cache = None


def _build_nc():
    global _nc_cache
    if _nc_cache is not None:
        return _nc_cache
    import concourse.tile as tile
    from concourse import bacc, mybir

    bdt = mybir.dt.bfloat16
    fdt = mybir.dt.float8e4
    nc = bacc.Bacc("TRN2", target_bir_lowering=False, debug=False)
    # partition-major DRAM layouts: one dma_start covers a GROUP of chunks
    # with large per-partition-contiguous descriptors.
    xT_ext = nc.dram_tensor("xT", [D, BC], bdt, kind="ExternalInput")
    rots_ext = None
    if not ONCHIP_ROT:
        rots_ext = nc.dram_tensor("rots", [D, NROT, BC], bdt,
                                  kind="ExternalInput")
    wb_ext = nc.dram_tensor("wb", [D, NBF, NOUT], bdt, kind="ExternalInput")
    w8_ext = None
    if M_PAIRS:
        w8_ext = nc.dram_tensor("w8", [D, M_PAIRS, 2, NOUT], fdt,
                                kind="ExternalInput")
    out_ext = nc.dram_tensor("out", [NOUT, BC], bdt, kind="ExternalOutput")

    # single-chunk groups through the DMA ramp window (the first ~10 chunks
    # are supply-latency-critical), larger groups once the wire has a lead.
    wg_sizes = _group_sizes(NBF, leading=(1, 1, 1, 1, 1, 2, 2), body=WB_BODY)
    rg_sizes = _group_sizes(NROT, leading=(1, 1, 1, 1, 1, 1, 2, 2, 2, 3, 3),
                            body=ROT_BODY)
    w8g_sizes = _group_sizes(M_PAIRS, leading=(2,), body=4) if M_PAIRS else []
    wg_starts = np.cumsum([0] + wg_sizes).tolist()
    rg_starts = np.cumsum([0] + rg_sizes).tolist()
    w8g_starts = np.cumsum([0] + w8g_sizes).tolist()

    def group_of(starts, i):
        for g in range(len(starts) - 1):
            if starts[g] <= i < starts[g + 1]:
                return g
        raise AssertionError

    # per-PROC-position metadata
    cross_pos = {c: k for k, c in enumerate(CROSS_ORDER)}   # rot slot
    bf16_pos = {}                                           # wb slot
    k = 0
    for c in PROC:
        if c not in _FP8_CHUNKS:
            bf16_pos[c] = k
            k += 1

    with tile.TileContext(nc) as tc:
        with (
            tc.tile_pool(name="xpool", bufs=1) as xpool,
            tc.tile_pool(name="wpool", bufs=4) as wpool,
            tc.tile_pool(name="w8pool", bufs=2) as w8pool,
            tc.tile_pool(name="rpool", bufs=8) as rpool,
            tc.tile_pool(name="ppool", bufs=8) as ppool,
            tc.tile_pool(name="p8pool", bufs=4) as p8pool,
            tc.tile_pool(name="opool", bufs=1) as opool,
            tc.tile_pool(name="psum", bufs=1, space="PSUM") as psum,
        ):
            ps = [[psum.tile([D, 512], mybir.dt.float32,
                             name=f"ps_{n}_{b}", tag=f"ps_{n}_{b}")
                   for b in range(NB)] for n in range(NN)]

            # ---- PE warmup: garbage matmuls to lift the HAM clock gate.
            # Rotate through the PSUM banks in the same order chunk 0's real
            # matmuls hit them, so each warmup only gates the bank whose real
            # matmul comes latest.
            if N_WARMUP:
                junk = xpool.tile([D, 512], bdt)
                nc.vector.memset(junk[:], 0.0)
                banks = [ps[n][b] for n in range(NN) for b in range(NB)]
                for i in range(N_WARMUP):
                    bank = banks[min(i, len(banks) - 1)]
                    nc.tensor.matmul(bank[:], junk[:, 0:128], junk[:],
                                     start=True, stop=True,
                                     skip_group_check=True)

            xT = xpool.tile([D, BC], bdt)

            wg_tiles = {}
            rg_tiles = {}
            w8g_tiles = {}

            def fetch_wb(slot):
                g = group_of(wg_starts, slot)
                if slot == wg_starts[g]:
                    sz = wg_sizes[g]
                    wg = wpool.tile([D, sz, NOUT], bdt, name="wg", tag="wg")
                    nc.sync.dma_start(wg[:], wb_ext[:, slot:slot + sz, :])
                    wg_tiles[g] = wg
                return wg_tiles[g], slot - wg_starts[g]

            def fetch_rot(slot):
                if ONCHIP_ROT:
                    # Build the rotation on-chip: rt[p] = xT[(p+d) % 128] via
                    # two SBUF->SBUF partition-shifted copies on the scalar
                    # HWDGE queue — its own descriptor generator and
                    # completion semaphore, so rot supply neither loads HBM
                    # nor couples to the weight stream's queue ordering.
                    if slot in rg_tiles:
                        return rg_tiles[slot][:]
                    dd = CROSS_ORDER[slot] - 1
                    rt = rpool.tile([D, BC], bdt, name="rt", tag="rt")
                    nc.scalar.dma_start(rt[0:D - dd, :], xT[dd:D, :])
                    nc.scalar.dma_start(rt[D - dd:D, :], xT[0:dd, :])
                    rg_tiles[slot] = rt
                    return rt[:]
                g = group_of(rg_starts, slot)
                if slot == rg_starts[g]:
                    sz = rg_sizes[g]
                    rg = rpool.tile([D, sz, BC], bdt, name="rg", tag="rg")
                    nc.sync.dma_start(rg[:], rots_ext[:, slot:slot + sz, :])
                    rg_tiles[g] = rg
                rg = rg_tiles[g]
                return rg[:, slot - rg_starts[g], :]

            def fetch_w8(pair):
                g = group_of(w8g_starts, pair)
                if pair == w8g_starts[g]:
                    sz = w8g_sizes[g]
                    wg = w8pool.tile([D, sz, 2, NOUT], fdt, name="w8g",
                                     tag="w8g")
                    nc.sync.dma_start(wg[:], w8_ext[:, pair:pair + sz, :, :])
                    w8g_tiles[g] = wg
                return w8g_tiles[g], pair - w8g_starts[g]

            # A consumer waits on the DMA queue's counting semaphore, i.e. on
            # ALL earlier-issued DMAs — so the head must issue strictly in
            # consumption order: chunk 0's weights, then each xT half with
            # its matmuls emitted before the next DMA is issued.
            first = PROC[0]
            last = PROC[-1]
            wg0, _ = fetch_wb(0)
            for b in range(NB):
                nc.sync.dma_start(xT[:, b * 512:(b + 1) * 512],
                                  xT_ext[:, b * 512:(b + 1) * 512])
                for n in range(NN):
                    nc.tensor.matmul(
                        ps[n][b][:],
                        wg0[:, 0, n * 128:(n + 1) * 128],
                        xT[:, b * 512:(b + 1) * 512],
                        start=True,
                        stop=False,
                    )

            # drain helpers: PSUM -> SBUF bf16 (scalar engine b=0, vector
            # b=1 so the two banks of a group copy in parallel), then one
            # DMA per group, alternated across the two HWDGE queues so
            # descriptor generation overlaps. Emitted INLINE with the last
            # chunk's matmuls so each bank's copy directly follows its
            # stop-matmul in program order (tightest PSUM-ready signaling).
            def drain_copy(n, b, ob):
                if b == 0:
                    nc.scalar.activation(
                        ob[:, 0:512], ps[n][0][:],
                        mybir.ActivationFunctionType.Identity)
                else:
                    nc.vector.tensor_copy(ob[:, 512:1024], ps[n][1][:])

            def drain_dma(n, ob):
                eng = nc.sync if n % 2 == 0 else nc.scalar
                eng.dma_start(
                    out_ext[n * 128:(n + 1) * 128, :]
                    .rearrange("p (b f) -> p b f", b=NB),
                    ob[:].rearrange("p (b f) -> p b f", b=NB),
                )

            obs = [opool.tile([D, NB * 512], bdt, name=f"ob{n}",
                              tag=f"ob{n}") for n in range(NN)]

            i = 1
            while i < len(PROC):
                c = PROC[i]
                if c in _FP8_CHUNKS:
                    pair = (i - (len(PROC) - 2 * M_PAIRS)) // 2
                    cA, cB = PROC[i], PROC[i + 1]
                    w8g, woff = fetch_w8(pair)
                    rsA = fetch_rot(cross_pos[cA])
                    rsB = fetch_rot(cross_pos[cB])
                    pp = p8pool.tile([D, 2, BC], fdt, name="p8", tag="p8")
                    nc.vector.tensor_mul(pp[:, 0, :], xT[:], rsA)
                    nc.vector.tensor_mul(pp[:, 1, :], xT[:], rsB)
                    stop = cB == last
                    for n in range(NN):
                        for b in range(NB):
                            nc.tensor.matmul(
                                ps[n][b][:],
                                w8g[:, woff, :, n * 128:(n + 1) * 128],
                                pp[:, :, b * 512:(b + 1) * 512],
                                start=(cA == first),
                                stop=stop,
                                perf_mode=mybir.MatmulPerfMode.DoubleRow,
                            )
                            if stop:
                                drain_copy(n, b, obs[n])
                        if stop:
                            drain_dma(n, obs[n])
                    i += 2
                    continue

                wg, woff = fetch_wb(bf16_pos[c])
                if c == 0:
                    mv = xT
                elif c == 1:
                    mv = ppool.tile([D, BC], bdt, name="pt", tag="pt")
                    nc.vector.tensor_mul(mv[:], xT[:], xT[:])
                else:
                    rslice = fetch_rot(cross_pos[c])
                    mv = ppool.tile([D, BC], bdt, name="pt", tag="pt")
                    nc.vector.tensor_mul(mv[:], xT[:], rslice)
                    if c == 65:
                        # all-ones feature row: its weight row carries bias.
                        # Same engine as the product, so no cross-engine
                        # semaphore hop before this chunk's matmuls.
                        nc.vector.memset(mv[BIAS_ROW:BIAS_ROW + 1, :], 1.0)
                stop = c == last
                for n in range(NN):
                    for b in range(NB):
                        nc.tensor.matmul(
                            ps[n][b][:],
                            wg[:, woff, n * 128:(n + 1) * 128],
                            mv[:, b * 512:(b + 1) * 512],
                            start=(c == first),
                            stop=stop,
                        )
                        if stop:
                            drain_copy(n, b, obs[n])
                    if stop:
                        drain_dma(n, obs[n])
                i += 1

    nc.compile()
    _nc_cache = nc
    return nc


def _prep_inputs(x, weights, bias):
    import ml_dtypes
    bdt_np = np.dtype(ml_dtypes.bfloat16)
    fdt_np = np.dtype(ml_dtypes.float8_e4m3)

    x = np.asarray(x, dtype=np.float32)
    weights = np.asarray(weights, dtype=np.float32)
    bias = np.asarray(bias, dtype=np.float32)

    idx = _pair_index_map()
    wcols = weights.T  # [8384, 512]
    wp = np.zeros((NCHUNK, D, NOUT), dtype=np.float32)
    valid = idx >= 0
    wp[valid] = wcols[idx[valid]]
    wp[65, BIAS_ROW, :] = bias  # ones-row bias fold (c65 row 64 is padding)

    wb = np.stack([wp[c] for c in PROC if c not in _FP8_CHUNKS])
    wb = np.ascontiguousarray(wb.transpose(1, 0, 2)).astype(bdt_np)
    w8 = None
    if M_PAIRS:
        w8 = np.stack([wp[c] for c in _FP8_CHUNKS])  # [2m, D, NOUT]
        w8 = w8.reshape(M_PAIRS, 2, D, NOUT).transpose(2, 0, 1, 3)
        w8 = np.ascontiguousarray(w8).astype(fdt_np)  # [D, m, 2, NOUT]

    in_maps = []
    for k in range(NCORES):
        xs = np.ascontiguousarray(x[k * BC:(k + 1) * BC].T).astype(bdt_np)
        im = {
            "xT": xs,
            "wb": wb,
        }
        if not ONCHIP_ROT:
            rots = np.stack(
                [np.roll(xs, -(c - 1), axis=0) for c in CROSS_ORDER])
            im["rots"] = np.ascontiguousarray(rots.transpose(1, 0, 2))
        if M_PAIRS:
            im["w8"] = w8
        in_maps.append(im)
    return in_maps


def kernel(x, weights, bias):
    _ensure_axon_hooks_stub()
    from concourse.bass_utils import run_bass_kernel_spmd

    nc = _build_nc()
    in_maps = _prep_inputs(x, weights, bias)
    res = run_bass_kernel_spmd(nc, in_maps, core_ids=list(range(NCORES)))
    outT = np.concatenate(
        [np.asarray(res.results[k]["out"]) for k in range(NCORES)], axis=1)
    out = np.ascontiguousarray(outT.T.astype(np.float32))  # [8192, 512]
    kernel.last_results = res
    return out
